# revision 7
# baseline (speedup 1.0000x reference)
"""Single-head causal attention (B=4, S=2048, D=1024, dk=128) on 8 TRN2 cores.

Sharding: core c -> batch b=c//2, half h=c%2.
  - h=0 handles query rows [0:512) u [1536:2048), h=1 handles [512:1536)
    (balances causal work: 4+16 vs 8+12 key-tiles per 512-query block).
  - Each core projects the full K/V for its batch (a collective exchange
    through the ncfw path measures ~36us on HW - not worth it).

Precision: qx/kx/wq/wk are fp8e4m3 (halves the score-path HBM bytes;
fp8 x fp8 matmuls run at bf16 rate), vx and wv stay bf16: fp8 on the V
path alone costs ~2.3e-2 max-rel error, over the 2e-2 budget.  wq/wk
are pre-scaled x16 into fp8's normal range; the x256 score inflation
is folded into the exp scale.  Output is stored bf16 (costs ~2e-3
rel).

The kernel is TENSOR-ENGINE-bound (~31us of matmul column-passes at
2.4 GHz warm) with a ~25us DMA load phase (7.75 MB/core at ~330 GB/s).
Schedule principles:
  - The first DMA wave is exactly the tensors that unblock the PE
    (wqk, qx halves, kx0/kx1); everything else queues behind.  vx3
    lands LAST and feeds the shortest possible post-landing chain
    (per-half V projection accumulation -> 4 transposes -> 16 PV
    matmuls -> div -> store).
  - Each kx piece feeds exactly one scores_pair for block 0 and one
    for block 1; score pairs are emitted in kx-arrival order so the
    serial 12-ACTIVATE exp chain (1.11us each) starts ~12us and never
    starves.
  - PV runs as two PSUM chains per 512-query block ([128, 2, 129]
    pair-tiles, 2 banks total): block 0 completes (divs + store)
    before block 1's chains claim the banks.
  - Warmup/filler matmuls use a 1-column stationary: they keep the HAM
    clock gate open without tripping the sustained-power throttle
    (k=4/8 HAM state halves the PE clock for ~5-7us when it fires).

Layout: the host pre-marshals every tensor into the exact [partition,
chunk, col] block layout the SBUF tiles use (2-8 KB contiguous runs
per partition per DMA).  Every DMA block and projection output gets
its own SBUF tile so tile-granular dependency tracking never
over-serializes.
"""

import math

import numpy as np
import ml_dtypes

import concourse.bacc as bacc
import concourse.tile as tile
import concourse.mybir as mybir
from concourse import bass_utils
from concourse.masks import make_identity
from concourse.tile_rust import add_dep_helper

F32 = mybir.dt.float32
BF16 = mybir.dt.bfloat16
FP8 = mybir.dt.float8e4

B, S, DM, DK = 4, 2048, 1024, 128
NCORES = 8
HALF = S // 2  # query rows per core
NCH = DM // 128  # d_model chunks
# program-wide causal shape: query block 0 sees key tiles [0, NJ0),
# block 1 sees [0, NJ1); per-core mask data zeroes what's invalid.
NJ0, NJ1 = 8, 16
VW = DK + 1  # v tiles carry a ones-column for the softmax denominator
WSC = 16.0  # wq/wk pre-scaled into fp8's normal range
SCALE = 1.0 / (math.sqrt(DK) * WSC * WSC)
WARMUP_MMS = 10
FILLER_MMS = 8

_CACHE = {}


def _build():
    if "nc" in _CACHE:
        return _CACHE["nc"]
    nc = bacc.Bacc("TRN2", target_bir_lowering=False, debug=False, num_devices=NCORES)

    # activations pre-blocked host-side to match SBUF tiles exactly
    qx_in = nc.dram_tensor("qx", [128, 2, NCH, 512], FP8, kind="ExternalInput").ap()
    kx_in = nc.dram_tensor("kx", [128, 8, NCH, 256], FP8, kind="ExternalInput").ap()
    vx_in = nc.dram_tensor("vx", [128, 4, NCH, 512], BF16, kind="ExternalInput").ap()
    wqk_in = nc.dram_tensor("wqk", [128, NCH, 2 * DK], FP8, kind="ExternalInput").ap()
    wv_in = nc.dram_tensor("wv", [128, NCH, DK], BF16, kind="ExternalInput").ap()
    shifts_in = nc.dram_tensor("shifts", [128, 16], F32, kind="ExternalInput").ap()
    out = nc.dram_tensor("out", [HALF, DK], BF16, kind="ExternalOutput").ap()

    with tile.TileContext(nc) as tc:
        with tc.tile_pool(name="const", bufs=1) as const:
            wqk = const.tile([128, NCH, 2 * DK], FP8, tag="wqk", name="wqk")
            wv = const.tile([128, NCH, DK], BF16, tag="wv", name="wv")
            shifts = const.tile([128, 16], F32)
            # qx halves: [block][half] -> chunks 4h..4h+3
            qxh = [[const.tile([128, 4, 512], FP8, tag=f"qx{b}{h}", name=f"qx{b}{h}")
                    for h in range(2)] for b in range(2)]
            kxp = [const.tile([128, NCH, 256], FP8, tag=f"kx{b}", name=f"kx{b}")
                   for b in range(8)]
            # vx: quarters 0/1 whole, quarters 2/3 split into chunk halves
            vx0 = const.tile([128, NCH, 512], BF16, tag="vx0", name="vx0")
            vx1 = const.tile([128, NCH, 512], BF16, tag="vx1", name="vx1")
            vxh = {(q, h): const.tile([128, 4, 512], BF16, tag=f"vx{q}{h}",
                                      name=f"vx{q}{h}")
                   for q in (2, 3) for h in range(2)}

            w_warm = const.tile([128, 512], BF16)
            nc.vector.memset(w_warm, 1.0)

            # ---- loads.  First wave = PE-unblocking tensors only.
            nc.scalar.dma_start(out=wqk, in_=wqk_in)
            nc.scalar.dma_start(out=kxp[2], in_=kx_in[:, 2])
            nc.scalar.dma_start(out=kxp[5], in_=kx_in[:, 5])
            nc.scalar.dma_start(out=vxh[2, 0], in_=vx_in[:, 2, 0:4])
            nc.scalar.dma_start(out=vxh[2, 1], in_=vx_in[:, 2, 4:8])

            nc.sync.dma_start(out=shifts, in_=shifts_in)
            nc.sync.dma_start(out=qxh[0][0], in_=qx_in[:, 0, 0:4])
            nc.sync.dma_start(out=qxh[0][1], in_=qx_in[:, 0, 4:8])
            nc.sync.dma_start(out=kxp[0], in_=kx_in[:, 0])
            nc.sync.dma_start(out=kxp[3], in_=kx_in[:, 3])
            nc.sync.dma_start(out=kxp[6], in_=kx_in[:, 6])
            nc.sync.dma_start(out=vx0, in_=vx_in[:, 0])
            nc.sync.dma_start(out=vxh[3, 0], in_=vx_in[:, 3, 0:4])

            nc.gpsimd.dma_start(out=qxh[1][0], in_=qx_in[:, 1, 0:4])
            nc.gpsimd.dma_start(out=qxh[1][1], in_=qx_in[:, 1, 4:8])
            nc.gpsimd.dma_start(out=kxp[1], in_=kx_in[:, 1])
            nc.gpsimd.dma_start(out=kxp[4], in_=kx_in[:, 4])
            nc.gpsimd.dma_start(out=kxp[7], in_=kx_in[:, 7])
            nc.gpsimd.dma_start(out=wv, in_=wv_in)
            nc.gpsimd.dma_start(out=vx1, in_=vx_in[:, 1])
            nc.gpsimd.dma_start(out=vxh[3, 1], in_=vx_in[:, 3, 4:8])

            # gpsimd auxiliary ops AFTER its dma issues (SWDGE descriptor
            # generation runs on the engine and must not be delayed)
            iota_i = const.tile([128, 1024], mybir.dt.int32)
            nc.gpsimd.iota(iota_i[:, 0:512], pattern=[[1, 512]], base=0,
                           channel_multiplier=0)
            nc.gpsimd.iota(iota_i[:, 512:1024], pattern=[[1, 512]], base=-128,
                           channel_multiplier=0)

            ident = const.tile([128, 128], BF16)
            make_identity(nc, ident)

            # ---- causal masks: mask[p, t, c] = (c >= shift[p, t]).
            iota2 = const.tile([128, 1024], mybir.dt.float16)
            nc.vector.tensor_copy(iota2, iota_i)
            masks_sb = {}

            def gen_masks(ts):
                for t in ts:
                    masks_sb[t] = const.tile([128, 1024], BF16, tag=f"mask{t}",
                                             name=f"mask{t}")
                    nc.vector.tensor_scalar(
                        masks_sb[t],
                        iota2,
                        shifts[:, t : t + 1],
                        None,
                        op0=mybir.AluOpType.is_ge,
                    )

            # ---- PE warmup + low-power fillers (1-col stationary)
            last_filler = None
            with tc.tile_pool(name="psW", bufs=1, space="PSUM") as psW:
                ps_w = psW.tile([128, 512], F32)
                for _ in range(WARMUP_MMS):
                    nc.tensor.matmul(
                        ps_w[:, 0:128], w_warm[:, 0:128], w_warm[:, 0:128],
                        start=True, stop=True
                    )
                for _ in range(FILLER_MMS):
                    last_filler = nc.tensor.matmul(
                        ps_w[0:1, :], w_warm[:, 0:1], w_warm, start=True, stop=True
                    )

            # ---- projected tensors: one tile per writer
            qTb = [const.tile([128, 512], BF16, tag=f"qT{b}", name=f"qT{b}")
                   for b in range(2)]
            kT = [const.tile([128, 256], BF16, tag=f"kT{b}", name=f"kT{b}")
                  for b in range(8)]
            vTq = [const.tile([128, 512], BF16, tag=f"vT{b}", name=f"vT{b}")
                   for b in range(4)]
            # vsb per quarter: [128, 4 key-tiles, VW]
            vsbq = [const.tile([128, 4, VW], BF16, tag=f"v{q}", name=f"vsb{q}")
                    for q in range(4)]

            with (
                tc.tile_pool(name="psM", bufs=2, space="PSUM") as psM,
                tc.tile_pool(name="psS", bufs=2, space="PSUM") as psS,
                tc.tile_pool(name="psO", bufs=2, space="PSUM") as psO,
                tc.tile_pool(name="pP", bufs=14) as p_pool,
                tc.tile_pool(name="oo", bufs=4) as o_pool,
            ):

                def dep_filler(mm):
                    if last_filler is not None:
                        add_dep_helper(mm.ins, last_filler.ins, sync=False,
                                       reason="run filler first")

                def project_dr(k0, parts, dst, w=512):
                    """fp8 DoubleRow projection of wqk[k0] against chunk-pair
                    list `parts` = [(tile, chunk_lo, n_chunks), ...]."""
                    acc = psM.tile([128, 512], F32, tag="ps_misc", name="acc")
                    u = 0
                    nu = NCH // 2
                    for t, c0, nch in parts:
                        for cc in range(0, nch, 2):
                            mm = nc.tensor.matmul(
                                acc[:, 0:w],
                                wqk[:, c0 + cc : c0 + cc + 2, k0 : k0 + DK],
                                t[:, cc : cc + 2, :],
                                start=(u == 0),
                                stop=(u == nu - 1),
                                perf_mode=mybir.MatmulPerfMode.DoubleRow,
                            )
                            if u == 0:
                                dep_filler(mm)
                            u += 1
                    nc.vector.tensor_copy(dst, acc[:, 0:w])

                def project_v(parts, dst):
                    """bf16 V projection (contract d_model chunks)."""
                    acc = psM.tile([128, 512], F32, tag="ps_misc", name="acc")
                    u = 0
                    for t, c0, nch in parts:
                        for cc in range(nch):
                            mm = nc.tensor.matmul(
                                acc,
                                wv[:, c0 + cc, :],
                                t[:, cc, :],
                                start=(u == 0),
                                stop=(u == NCH - 1),
                            )
                            if u == 0:
                                dep_filler(mm)
                            u += 1
                    nc.vector.tensor_copy(dst, acc)

                def scores_pair(blk, j, masked):
                    """exp(score) for key tiles (j, j+1) x 512 queries of blk."""
                    ps_s = psS.tile([128, 1024], F32, tag="score")
                    for i in range(2):
                        jl = j + i
                        nc.tensor.matmul(
                            ps_s[:, i * 512 : (i + 1) * 512],
                            kT[jl // 2][:, (jl % 2) * 128 : (jl % 2 + 1) * 128],
                            qTb[blk],
                            start=True,
                            stop=True,
                        )
                    p_t = p_pool.tile([128, 1024], BF16, tag="p")
                    nc.scalar.activation(
                        p_t, ps_s, mybir.ActivationFunctionType.Exp, scale=SCALE
                    )
                    if masked:
                        nc.vector.tensor_mul(p_t, p_t, masks_sb[j])
                    return p_t

                def v_quarter(q, parts):
                    """project + transpose vx quarter q into vsbq[q]."""
                    nc.vector.memset(vsbq[q][:, :, DK : DK + 1], 1.0)
                    project_v(parts, vTq[q])
                    ps = psM.tile([128, 4, 128], BF16, tag="ps_misc")
                    for tl in range(4):
                        nc.tensor.transpose(
                            ps[:, tl, :], vTq[q][:, tl * 128 : (tl + 1) * 128],
                            ident
                        )
                    nc.vector.tensor_copy(vsbq[q][:, :, 0:DK], ps)

                o_big = [
                    o_pool.tile([128, 4, DK], BF16, tag=f"ob{b}", name=f"ob{b}",
                                bufs=1)
                    for b in range(2)
                ]
                out4 = out.rearrange("(b p q) k -> b p q k", q=4, p=128)

                def pv(ps_o, p_pairs, qs, jset, start, stop):
                    """accumulate P@[V|1] for query-slice qs over key tiles."""
                    for n, j in enumerate(jset):
                        nc.tensor.matmul(
                            ps_o,
                            p_pairs[(j // 2) * 2][
                                :, (j % 2) * 512 + qs * 128
                                : (j % 2) * 512 + (qs + 1) * 128
                            ],
                            vsbq[j // 4][:, j % 4, :],
                            start=(start and n == 0),
                            stop=(stop and n == len(jset) - 1),
                        )

                def div_out(blk, qs, ps_o):
                    rec = o_pool.tile([128, 1], F32, tag="rec")
                    nc.vector.reciprocal(rec, ps_o[:, DK : DK + 1])
                    nc.vector.tensor_scalar_mul(
                        o_big[blk][:, qs, :], ps_o[:, 0:DK], rec
                    )

                # ---------- pipeline ----------
                # W hints = predicted data-ready times (us), pacing the
                # static scheduler's per-engine instruction order.
                W = tc.tile_wait_until
                with W(0.0095):
                    project_dr(0, [(qxh[0][0], 0, 4), (qxh[0][1], 4, 4)], qTb[0])
                    gen_masks([0, 2])
                with W(0.0105):
                    project_dr(DK, [(kxp[0], 0, NCH)], kT[0], w=256)
                with W(0.011):
                    project_dr(0, [(qxh[1][0], 0, 4), (qxh[1][1], 4, 4)], qTb[1])
                p0, p1, p1b = {}, {}, {}
                with W(0.0116):
                    p0[0] = scores_pair(0, 0, True)
                with W(0.012):
                    project_dr(DK, [(kxp[1], 0, NCH)], kT[1], w=256)
                with W(0.0124):
                    p1[0] = scores_pair(1, 0, False)
                    gen_masks([4, 6])
                with W(0.0128):
                    project_dr(DK, [(kxp[2], 0, NCH)], kT[2], w=256)
                with W(0.013):
                    p0[2] = scores_pair(0, 2, True)
                with W(0.0135):
                    p1[2] = scores_pair(1, 2, False)
                    project_dr(DK, [(kxp[3], 0, NCH)], kT[3], w=256)
                with W(0.014):
                    p0[4] = scores_pair(0, 4, True)
                with W(0.0145):
                    v_quarter(0, [(vx0, 0, NCH)])
                with W(0.015):
                    p1[4] = scores_pair(1, 4, False)
                    project_dr(DK, [(kxp[4], 0, NCH)], kT[4], w=256)
                    gen_masks([8, 10])
                with W(0.016):
                    p0[6] = scores_pair(0, 6, True)
                with W(0.0165):
                    p1[6] = scores_pair(1, 6, False)
                    project_dr(DK, [(kxp[5], 0, NCH)], kT[5], w=256)
                with W(0.017):
                    p1b[8] = scores_pair(1, 8, True)
                    gen_masks([12, 14])
                with W(0.0175):
                    v_quarter(1, [(vx1, 0, NCH)])
                with W(0.018):
                    p1b[10] = scores_pair(1, 10, True)
                    project_dr(DK, [(kxp[6], 0, NCH)], kT[6], w=256)
                with W(0.0185):
                    p1b[12] = scores_pair(1, 12, True)
                    project_dr(DK, [(kxp[7], 0, NCH)], kT[7], w=256)
                with W(0.019):
                    p1b[14] = scores_pair(1, 14, True)

                # block 0: PV chains, divs, store (frees psO banks for blk 1)
                ps_o0 = [psO.tile([128, VW], F32, tag="out", name=f"ps_o0_{i}")
                         for i in range(4)]
                with W(0.021):
                    for qs in range(4):
                        pv(ps_o0[qs], p0, qs, range(NJ0), True, True)
                        div_out(0, qs, ps_o0[qs])
                    nc.sync.dma_start(out=out4[0], in_=o_big[0])
                with W(0.022):
                    v_quarter(2, [(vxh[2, 0], 0, 4), (vxh[2, 1], 4, 4)])

                # block 1: one long accumulation per query-slice
                ps_o1 = [psO.tile([128, VW], F32, tag="out", name=f"ps_o1_{i}")
                         for i in range(4)]
                with W(0.0245):
                    for qs in range(4):
                        pv(ps_o1[qs], p1, qs, range(NJ0), True, False)
                with W(0.026):
                    for qs in range(4):
                        pv(ps_o1[qs], p1b, qs, range(8, 12), False, False)
                with W(0.028):
                    v_quarter(3, [(vxh[3, 0], 0, 4), (vxh[3, 1], 4, 4)])
                with W(0.031):
                    for qs in range(4):
                        pv(ps_o1[qs], p1b, qs, range(12, NJ1), False, True)
                        div_out(1, qs, ps_o1[qs])
                with W(0.033):
                    nc.scalar.dma_start(out=out4[1, :, 0:2], in_=o_big[1][:, 0:2])
                    nc.sync.dma_start(out=out4[1, :, 2:4], in_=o_big[1][:, 2:4])

    nc.compile()
    _CACHE["nc"] = nc
    return nc


def _shift_block(h):
    """[128, 16] f32: mask[p, t, c] = (c >= shift) == (key 128t+p <= query qb+c)."""
    qbase = (0, 1536) if h == 0 else (512, 1024)
    p = np.arange(128, dtype=np.float32)[:, None]
    t = np.arange(16, dtype=np.float32)[None, :]
    qb = np.where(t < NJ0, qbase[0], qbase[1])
    return (128.0 * t + p - qb).astype(np.float32)


def _blocked(arr, nblk, dtype):
    """[DM, ncols] -> [128, nblk, NCH, ncols//nblk] matching the SBUF tiles."""
    w = arr.shape[1] // nblk
    return np.ascontiguousarray(
        arr.reshape(NCH, 128, nblk, w).transpose(1, 2, 0, 3)
    ).astype(dtype)


def kernel(**inputs):
    queries = np.asarray(inputs["queries"], dtype=np.float32)
    keys = np.asarray(inputs["keys"], dtype=np.float32)
    values = np.asarray(inputs["values"], dtype=np.float32)

    nc = _build()
    f8 = ml_dtypes.float8_e4m3fn
    bf = ml_dtypes.bfloat16
    shifts = [_shift_block(0), _shift_block(1)]
    qrows = [np.r_[0:512, 1536:2048], np.r_[512:1536]]
    wT = {
        nm: np.asarray(inputs[nm], dtype=np.float32).T
        for nm in ("Wq", "Wk", "Wv")
    }
    wqk = np.ascontiguousarray(
        np.concatenate([wT["Wq"], wT["Wk"]], axis=1).reshape(NCH, 128, 2 * DK)
        .transpose(1, 0, 2) * WSC
    ).astype(f8)
    wv = np.ascontiguousarray(
        wT["Wv"].reshape(NCH, 128, DK).transpose(1, 0, 2)
    ).astype(bf)
    kxs = [_blocked(keys[b].T, 8, f8) for b in range(B)]
    vxs = [_blocked(values[b].T, 4, bf) for b in range(B)]

    in_maps = []
    for c in range(NCORES):
        b, h = c // 2, c % 2
        in_maps.append(
            {
                "qx": _blocked(queries[b][qrows[h]].T, 2, f8),
                "kx": kxs[b],
                "vx": vxs[b],
                "wqk": wqk,
                "wv": wv,
                "shifts": shifts[h],
            }
        )

    res = bass_utils.run_bass_kernel_spmd(
        nc, in_maps, list(range(NCORES)), **_CACHE.get("run_kwargs", {})
    )
    _CACHE["last_result"] = res

    # store layout is (p q): dram row blk*512 + p*4 + qs <- query qs*128 + p
    r = np.arange(512)
    local_q = (r % 4) * 128 + r // 4  # query index within block at dram row r
    perm = np.concatenate([local_q, 512 + local_q])
    out = np.empty((B, S, DK), dtype=np.float32)
    for c in range(NCORES):
        b, h = c // 2, c % 2
        out[b][qrows[h][perm]] = res.results[c]["out"]
    return out


# revision 10
# speedup vs baseline: 1.1422x; 1.1422x over previous
"""Single-head causal attention (B=4, S=2048, D=1024, dk=128) on 8 TRN2 cores.

Sharding: core c -> batch b=c//2, half h=c%2.
  - h=0 handles query rows [0:512) u [1536:2048), h=1 handles [512:1536)
    (balances causal work: 4+16 vs 8+12 key-tiles per 512-query block).
  - Each core projects the full K/V for its batch (a collective exchange
    through the ncfw path measures ~36us on HW - not worth it).

Precision: qx/kx/wq/wk are fp8e4m3 (halves the score-path HBM bytes;
fp8 x fp8 matmuls run at bf16 rate), vx and wv stay bf16: fp8 on the V
path alone costs ~2.3e-2 max-rel error, over the 2e-2 budget.  wq/wk
are pre-scaled x16 into fp8's normal range; the x256 score inflation
is folded into the exp scale.  Output is stored bf16 (costs ~2e-3
rel).

The kernel is TENSOR-ENGINE-bound (~31us of matmul column-passes at
2.4 GHz warm) with a ~25us DMA load phase (7.75 MB/core at ~330 GB/s).
Schedule principles:
  - The first DMA wave is exactly the tensors that unblock the PE
    (wqk, qx halves, kx0/kx1); everything else queues behind.  vx3
    lands LAST and feeds the shortest possible post-landing chain
    (per-half V projection accumulation -> 4 transposes -> 16 PV
    matmuls -> div -> store).
  - Each kx piece feeds exactly one scores_pair for block 0 and one
    for block 1; score pairs are emitted in kx-arrival order so the
    serial 12-ACTIVATE exp chain (1.11us each) starts ~12us and never
    starves.
  - PV runs as two PSUM chains per 512-query block ([128, 2, 129]
    pair-tiles, 2 banks total): block 0 completes (divs + store)
    before block 1's chains claim the banks.
  - Warmup/filler matmuls use a 1-column stationary: they keep the HAM
    clock gate open without tripping the sustained-power throttle
    (k=4/8 HAM state halves the PE clock for ~5-7us when it fires).

Layout: the host pre-marshals every tensor into the exact [partition,
chunk, col] block layout the SBUF tiles use (2-8 KB contiguous runs
per partition per DMA).  Every DMA block and projection output gets
its own SBUF tile so tile-granular dependency tracking never
over-serializes.
"""

import math

import numpy as np
import ml_dtypes

import concourse.bacc as bacc
import concourse.tile as tile
import concourse.mybir as mybir
from concourse import bass_utils
from concourse.masks import make_identity
from concourse.tile_rust import add_dep_helper

F32 = mybir.dt.float32
BF16 = mybir.dt.bfloat16
FP8 = mybir.dt.float8e4

B, S, DM, DK = 4, 2048, 1024, 128
NCORES = 8
HALF = S // 2  # query rows per core
NCH = DM // 128  # d_model chunks
# program-wide causal shape: query block 0 sees key tiles [0, NJ0),
# block 1 sees [0, NJ1); per-core mask data zeroes what's invalid.
NJ0, NJ1 = 8, 16
VW = DK + 1  # v tiles carry a ones-column for the softmax denominator
WSC = 16.0  # wq/wk pre-scaled into fp8's normal range
SCALE = 1.0 / (math.sqrt(DK) * WSC * WSC)
WARMUP_MMS = 10
FILLER_MMS = 8

_CACHE = {}


def _build():
    if "nc" in _CACHE:
        return _CACHE["nc"]
    nc = bacc.Bacc("TRN2", target_bir_lowering=False, debug=False, num_devices=NCORES)

    # activations pre-blocked host-side to match SBUF tiles exactly
    qx_in = nc.dram_tensor("qx", [128, 2, NCH, 512], FP8, kind="ExternalInput").ap()
    kx_in = nc.dram_tensor("kx", [128, 8, NCH, 256], FP8, kind="ExternalInput").ap()
    vx_in = nc.dram_tensor("vx", [128, 4, NCH, 512], BF16, kind="ExternalInput").ap()
    wqk_in = nc.dram_tensor("wqk", [128, NCH, 2 * DK], FP8, kind="ExternalInput").ap()
    wv_in = nc.dram_tensor("wv", [128, NCH, DK], BF16, kind="ExternalInput").ap()
    shifts_in = nc.dram_tensor("shifts", [128, 16], F32, kind="ExternalInput").ap()
    out = nc.dram_tensor("out", [HALF, DK], BF16, kind="ExternalOutput").ap()

    with tile.TileContext(nc) as tc:
        with tc.tile_pool(name="const", bufs=1) as const:
            wqk = const.tile([128, NCH, 2 * DK], FP8, tag="wqk", name="wqk")
            wv = const.tile([128, NCH, DK], BF16, tag="wv", name="wv")
            shifts = const.tile([128, 16], F32)
            # qx halves: [block][half] -> chunks 4h..4h+3
            qxh = [[const.tile([128, 4, 512], FP8, tag=f"qx{b}{h}", name=f"qx{b}{h}")
                    for h in range(2)] for b in range(2)]
            kxp = [const.tile([128, NCH, 256], FP8, tag=f"kx{b}", name=f"kx{b}")
                   for b in range(8)]
            # vx: every quarter split into two chunk-half tiles so the
            # projection can consume each half as it lands
            vxh = {(q, h): const.tile([128, 4, 512], BF16, tag=f"vx{q}{h}",
                                      name=f"vx{q}{h}")
                   for q in range(4) for h in range(2)}

            w_warm = const.tile([128, 512], BF16)
            nc.vector.memset(w_warm, 1.0)

            # ---- loads.  Each queue drains at ~110 GB/s, so an item's
            # landing time is ~8.4us + cumulative-bytes-before-it/110GB/s.
            # Queue positions are chosen so every tensor lands just before
            # its consumer needs it; the exp chain (one kx piece per
            # ~2.2us from ~14us) and the vx quarters (one per ~4.5us from
            # ~20us) set the deadlines.  vx3 lands last by design.
            # scalar: 5 up-front (HWDGE ring holds 4; the 5th blocks until
            # wqk completes ~10.7) + vx2a/kx7 issued mid-exp-chain.
            nc.scalar.dma_start(out=wqk, in_=wqk_in)
            nc.scalar.dma_start(out=qxh[0][1], in_=qx_in[:, 0, 4:8])
            nc.scalar.dma_start(out=kxp[2], in_=kx_in[:, 2])
            nc.scalar.dma_start(out=vxh[0, 0], in_=vx_in[:, 0, 0:4])
            nc.scalar.dma_start(out=vxh[1, 0], in_=vx_in[:, 1, 0:4])

            nc.sync.dma_start(out=shifts, in_=shifts_in)
            nc.sync.dma_start(out=qxh[0][0], in_=qx_in[:, 0, 0:4])
            nc.sync.dma_start(out=qxh[1][1], in_=qx_in[:, 1, 4:8])
            nc.sync.dma_start(out=kxp[3], in_=kx_in[:, 3])
            nc.sync.dma_start(out=wv, in_=wv_in)
            nc.sync.dma_start(out=vxh[0, 1], in_=vx_in[:, 0, 4:8])
            nc.sync.dma_start(out=vxh[2, 1], in_=vx_in[:, 2, 4:8])
            nc.sync.dma_start(out=vxh[3, 0], in_=vx_in[:, 3, 0:4])

            nc.gpsimd.dma_start(out=kxp[0], in_=kx_in[:, 0])
            nc.gpsimd.dma_start(out=qxh[1][0], in_=qx_in[:, 1, 0:4])
            nc.gpsimd.dma_start(out=kxp[1], in_=kx_in[:, 1])
            nc.gpsimd.dma_start(out=kxp[4], in_=kx_in[:, 4])
            nc.gpsimd.dma_start(out=kxp[5], in_=kx_in[:, 5])
            nc.gpsimd.dma_start(out=vxh[1, 1], in_=vx_in[:, 1, 4:8])
            nc.gpsimd.dma_start(out=kxp[6], in_=kx_in[:, 6])
            nc.gpsimd.dma_start(out=vxh[3, 1], in_=vx_in[:, 3, 4:8])

            # gpsimd auxiliary ops AFTER its dma issues (SWDGE descriptor
            # generation runs on the engine and must not be delayed)
            iota_i = const.tile([128, 1024], mybir.dt.int32)
            nc.gpsimd.iota(iota_i[:, 0:512], pattern=[[1, 512]], base=0,
                           channel_multiplier=0)
            nc.gpsimd.iota(iota_i[:, 512:1024], pattern=[[1, 512]], base=-128,
                           channel_multiplier=0)

            ident = const.tile([128, 128], BF16)
            make_identity(nc, ident)

            # ---- causal masks: mask[p, t, c] = (c >= shift[p, t]).
            # iota2's copy is emitted mid-pipeline: it waits on the gpsimd
            # iotas (ready ~15.3us after the lib load) and must not block
            # the kT/qT casts in the in-order vector stream.
            iota2 = const.tile([128, 1024], mybir.dt.float16)
            masks_sb = {}

            def gen_masks(ts):
                for t in ts:
                    masks_sb[t] = const.tile([128, 1024], BF16, tag=f"mask{t}",
                                             name=f"mask{t}")
                    nc.vector.tensor_scalar(
                        masks_sb[t],
                        iota2,
                        shifts[:, t : t + 1],
                        None,
                        op0=mybir.AluOpType.is_ge,
                    )

            # ---- PE warmup + low-power fillers (1-col stationary)
            last_filler = None
            with tc.tile_pool(name="psW", bufs=1, space="PSUM") as psW:
                ps_w = psW.tile([128, 512], F32)
                for _ in range(WARMUP_MMS):
                    nc.tensor.matmul(
                        ps_w[:, 0:128], w_warm[:, 0:128], w_warm[:, 0:128],
                        start=True, stop=True
                    )
                for _ in range(FILLER_MMS):
                    last_filler = nc.tensor.matmul(
                        ps_w[0:1, :], w_warm[:, 0:1], w_warm, start=True, stop=True
                    )

            # ---- projected tensors: one tile per writer
            qTb = [const.tile([128, 512], BF16, tag=f"qT{b}", name=f"qT{b}")
                   for b in range(2)]
            kT = [const.tile([128, 256], BF16, tag=f"kT{b}", name=f"kT{b}")
                  for b in range(8)]
            vTq = [const.tile([128, 512], BF16, tag=f"vT{b}", name=f"vT{b}")
                   for b in range(4)]
            # vsb per quarter: [128, 4 key-tiles, VW]
            vsbq = [const.tile([128, 4, VW], BF16, tag=f"v{q}", name=f"vsb{q}")
                    for q in range(4)]

            with (
                tc.tile_pool(name="psM", bufs=2, space="PSUM") as psM,
                tc.tile_pool(name="psS", bufs=2, space="PSUM") as psS,
                tc.tile_pool(name="psO", bufs=2, space="PSUM") as psO,
                tc.tile_pool(name="pP", bufs=14) as p_pool,
                tc.tile_pool(name="oo", bufs=4) as o_pool,
            ):

                def dep_filler(mm):
                    if last_filler is not None:
                        add_dep_helper(mm.ins, last_filler.ins, sync=False,
                                       reason="run filler first")

                def project_dr(k0, parts, dst, w=512):
                    """fp8 DoubleRow projection of wqk[k0] against chunk-pair
                    list `parts` = [(tile, chunk_lo, n_chunks), ...]."""
                    acc = psM.tile([128, 512], F32, tag="ps_misc", name="acc")
                    u = 0
                    nu = NCH // 2
                    for t, c0, nch in parts:
                        for cc in range(0, nch, 2):
                            mm = nc.tensor.matmul(
                                acc[:, 0:w],
                                wqk[:, c0 + cc : c0 + cc + 2, k0 : k0 + DK],
                                t[:, cc : cc + 2, :],
                                start=(u == 0),
                                stop=(u == nu - 1),
                                perf_mode=mybir.MatmulPerfMode.DoubleRow,
                            )
                            if u == 0:
                                dep_filler(mm)
                            u += 1
                    nc.vector.tensor_copy(dst, acc[:, 0:w])

                def project_v(parts, dst):
                    """bf16 V projection (contract d_model chunks)."""
                    acc = psM.tile([128, 512], F32, tag="ps_misc", name="acc")
                    u = 0
                    for t, c0, nch in parts:
                        for cc in range(nch):
                            mm = nc.tensor.matmul(
                                acc,
                                wv[:, c0 + cc, :],
                                t[:, cc, :],
                                start=(u == 0),
                                stop=(u == NCH - 1),
                            )
                            if u == 0:
                                dep_filler(mm)
                            u += 1
                    nc.vector.tensor_copy(dst, acc)

                def scores_pair(blk, j, masked):
                    """exp(score) for key tiles (j, j+1) x 512 queries of blk."""
                    ps_s = psS.tile([128, 1024], F32, tag="score")
                    for i in range(2):
                        jl = j + i
                        nc.tensor.matmul(
                            ps_s[:, i * 512 : (i + 1) * 512],
                            kT[jl // 2][:, (jl % 2) * 128 : (jl % 2 + 1) * 128],
                            qTb[blk],
                            start=True,
                            stop=True,
                        )
                    p_t = p_pool.tile([128, 1024], BF16, tag="p")
                    nc.scalar.activation(
                        p_t, ps_s, mybir.ActivationFunctionType.Exp, scale=SCALE
                    )
                    if masked:
                        nc.vector.tensor_mul(p_t, p_t, masks_sb[j])
                    return p_t

                def v_quarter(q, parts):
                    """project + transpose vx quarter q into vsbq[q]."""
                    nc.vector.memset(vsbq[q][:, :, DK : DK + 1], 1.0)
                    project_v(parts, vTq[q])
                    ps = psM.tile([128, 4, 128], BF16, tag="ps_misc")
                    for tl in range(4):
                        nc.tensor.transpose(
                            ps[:, tl, :], vTq[q][:, tl * 128 : (tl + 1) * 128],
                            ident
                        )
                    nc.vector.tensor_copy(vsbq[q][:, :, 0:DK], ps)

                o_big = [
                    o_pool.tile([128, 4, DK], BF16, tag=f"ob{b}", name=f"ob{b}",
                                bufs=1)
                    for b in range(2)
                ]
                out4 = out.rearrange("(b p q) k -> b p q k", q=4, p=128)

                def pv(ps_o, p_pairs, qs, jset, start, stop):
                    """accumulate P@[V|1] for query-slice qs over key tiles."""
                    for n, j in enumerate(jset):
                        nc.tensor.matmul(
                            ps_o,
                            p_pairs[(j // 2) * 2][
                                :, (j % 2) * 512 + qs * 128
                                : (j % 2) * 512 + (qs + 1) * 128
                            ],
                            vsbq[j // 4][:, j % 4, :],
                            start=(start and n == 0),
                            stop=(stop and n == len(jset) - 1),
                        )

                def div_out(blk, qs, ps_o):
                    rec = o_pool.tile([128, 1], F32, tag="rec")
                    nc.vector.reciprocal(rec, ps_o[:, DK : DK + 1])
                    nc.vector.tensor_scalar_mul(
                        o_big[blk][:, qs, :], ps_o[:, 0:DK], rec
                    )

                # ---------- pipeline ----------
                # W hints = predicted execution times (us) from the DMA
                # landing model; they shape each engine's static order.
                W = tc.tile_wait_until
                with W(0.0105):
                    project_dr(DK, [(kxp[0], 0, NCH)], kT[0], w=256)
                with W(0.011):
                    project_dr(0, [(qxh[0][0], 0, 4), (qxh[0][1], 4, 4)], qTb[0])
                with W(0.0115):
                    project_dr(0, [(qxh[1][0], 0, 4), (qxh[1][1], 4, 4)], qTb[1])
                with W(0.012):
                    nc.vector.tensor_copy(iota2, iota_i)
                    gen_masks([0, 2])
                p0, p1, p1b = {}, {}, {}
                with W(0.013):
                    p0[0] = scores_pair(0, 0, True)
                with W(0.0135):
                    p1[0] = scores_pair(1, 0, False)
                with W(0.014):
                    project_dr(DK, [(kxp[2], 0, NCH)], kT[2], w=256)
                    project_dr(DK, [(kxp[1], 0, NCH)], kT[1], w=256)
                with W(0.0145):
                    p0[2] = scores_pair(0, 2, True)
                    gen_masks([4, 6])
                with W(0.015):
                    p1[2] = scores_pair(1, 2, False)
                with W(0.0155):
                    project_dr(DK, [(kxp[3], 0, NCH)], kT[3], w=256)
                with W(0.016):
                    p0[4] = scores_pair(0, 4, True)
                with W(0.0165):
                    p1[4] = scores_pair(1, 4, False)
                    # mid-chain scalar issues (queue tail; land ~29/31)
                    nc.scalar.dma_start(out=vxh[2, 0], in_=vx_in[:, 2, 0:4])
                with W(0.017):
                    p0[6] = scores_pair(0, 6, True)
                    gen_masks([8, 10])
                with W(0.0175):
                    p1[6] = scores_pair(1, 6, False)
                    nc.scalar.dma_start(out=kxp[7], in_=kx_in[:, 7])
                with W(0.018):
                    project_dr(DK, [(kxp[4], 0, NCH)], kT[4], w=256)
                with W(0.0185):
                    p1b[8] = scores_pair(1, 8, True)
                    gen_masks([12, 14])
                with W(0.020):
                    v_quarter(0, [(vxh[0, 0], 0, 4), (vxh[0, 1], 4, 4)])
                with W(0.0205):
                    project_dr(DK, [(kxp[5], 0, NCH)], kT[5], w=256)
                with W(0.021):
                    p1b[10] = scores_pair(1, 10, True)

                # block 0: PV chains, divs, store (frees psO banks for blk 1)
                ps_o0 = [psO.tile([128, VW], F32, tag="out", name=f"ps_o0_{i}")
                         for i in range(4)]
                with W(0.0245):
                    for qs in range(4):
                        pv(ps_o0[qs], p0, qs, range(NJ0), True, True)
                        div_out(0, qs, ps_o0[qs])
                    nc.sync.dma_start(out=out4[0], in_=o_big[0])
                with W(0.0255):
                    v_quarter(1, [(vxh[1, 0], 0, 4), (vxh[1, 1], 4, 4)])

                # block 1: one long accumulation per query-slice
                ps_o1 = [psO.tile([128, VW], F32, tag="out", name=f"ps_o1_{i}")
                         for i in range(4)]
                with W(0.0275):
                    for qs in range(4):
                        pv(ps_o1[qs], p1, qs, range(NJ0), True, False)
                with W(0.0278):
                    project_dr(DK, [(kxp[6], 0, NCH)], kT[6], w=256)
                with W(0.0285):
                    p1b[12] = scores_pair(1, 12, True)
                with W(0.029):
                    v_quarter(2, [(vxh[2, 0], 0, 4), (vxh[2, 1], 4, 4)])
                with W(0.0305):
                    for qs in range(4):
                        pv(ps_o1[qs], p1b, qs, range(8, 12), False, False)
                with W(0.0315):
                    project_dr(DK, [(kxp[7], 0, NCH)], kT[7], w=256)
                with W(0.032):
                    p1b[14] = scores_pair(1, 14, True)
                with W(0.0325):
                    v_quarter(3, [(vxh[3, 0], 0, 4), (vxh[3, 1], 4, 4)])
                with W(0.034):
                    for qs in range(4):
                        pv(ps_o1[qs], p1b, qs, range(12, NJ1), False, True)
                        div_out(1, qs, ps_o1[qs])
                with W(0.035):
                    nc.scalar.dma_start(out=out4[1, :, 0:2], in_=o_big[1][:, 0:2])
                    nc.sync.dma_start(out=out4[1, :, 2:4], in_=o_big[1][:, 2:4])

    nc.compile()
    _CACHE["nc"] = nc
    return nc


def _shift_block(h):
    """[128, 16] f32: mask[p, t, c] = (c >= shift) == (key 128t+p <= query qb+c)."""
    qbase = (0, 1536) if h == 0 else (512, 1024)
    p = np.arange(128, dtype=np.float32)[:, None]
    t = np.arange(16, dtype=np.float32)[None, :]
    qb = np.where(t < NJ0, qbase[0], qbase[1])
    return (128.0 * t + p - qb).astype(np.float32)


def _blocked(arr, nblk, dtype):
    """[DM, ncols] -> [128, nblk, NCH, ncols//nblk] matching the SBUF tiles."""
    w = arr.shape[1] // nblk
    return np.ascontiguousarray(
        arr.reshape(NCH, 128, nblk, w).transpose(1, 2, 0, 3)
    ).astype(dtype)


def kernel(**inputs):
    queries = np.asarray(inputs["queries"], dtype=np.float32)
    keys = np.asarray(inputs["keys"], dtype=np.float32)
    values = np.asarray(inputs["values"], dtype=np.float32)

    nc = _build()
    f8 = ml_dtypes.float8_e4m3fn
    bf = ml_dtypes.bfloat16
    shifts = [_shift_block(0), _shift_block(1)]
    qrows = [np.r_[0:512, 1536:2048], np.r_[512:1536]]
    wT = {
        nm: np.asarray(inputs[nm], dtype=np.float32).T
        for nm in ("Wq", "Wk", "Wv")
    }
    wqk = np.ascontiguousarray(
        np.concatenate([wT["Wq"], wT["Wk"]], axis=1).reshape(NCH, 128, 2 * DK)
        .transpose(1, 0, 2) * WSC
    ).astype(f8)
    wv = np.ascontiguousarray(
        wT["Wv"].reshape(NCH, 128, DK).transpose(1, 0, 2)
    ).astype(bf)
    kxs = [_blocked(keys[b].T, 8, f8) for b in range(B)]
    vxs = [_blocked(values[b].T, 4, bf) for b in range(B)]

    in_maps = []
    for c in range(NCORES):
        b, h = c // 2, c % 2
        in_maps.append(
            {
                "qx": _blocked(queries[b][qrows[h]].T, 2, f8),
                "kx": kxs[b],
                "vx": vxs[b],
                "wqk": wqk,
                "wv": wv,
                "shifts": shifts[h],
            }
        )

    res = bass_utils.run_bass_kernel_spmd(
        nc, in_maps, list(range(NCORES)), **_CACHE.get("run_kwargs", {})
    )
    _CACHE["last_result"] = res

    # store layout is (p q): dram row blk*512 + p*4 + qs <- query qs*128 + p
    r = np.arange(512)
    local_q = (r % 4) * 128 + r // 4  # query index within block at dram row r
    perm = np.concatenate([local_q, 512 + local_q])
    out = np.empty((B, S, DK), dtype=np.float32)
    for c in range(NCORES):
        b, h = c // 2, c % 2
        out[b][qrows[h][perm]] = res.results[c]["out"]
    return out


# revision 13
# speedup vs baseline: 1.1921x; 1.0437x over previous
"""Single-head causal attention (B=4, S=2048, D=1024, dk=128) on 8 TRN2 cores.

Sharding: core c -> batch b=c//2, half h=c%2.
  - h=0 handles query rows [0:512) u [1536:2048), h=1 handles [512:1536)
    (balances causal work: 4+16 vs 8+12 key-tiles per 512-query block).
  - Each core projects the full K/V for its batch (a collective exchange
    through the ncfw path measures ~36us on HW - not worth it).

Precision: qx/kx/wq/wk are fp8e4m3 (halves the score-path HBM bytes;
fp8 x fp8 matmuls run at bf16 rate), vx and wv stay bf16: fp8 on the V
path alone costs ~2.3e-2 max-rel error, over the 2e-2 budget.  wq/wk
are pre-scaled x16 into fp8's normal range; the x256 score inflation
is folded into the exp scale.  Output is stored bf16 (costs ~2e-3
rel).

The kernel is TENSOR-ENGINE-bound (~31us of matmul column-passes at
2.4 GHz warm) with a ~25us DMA load phase (7.75 MB/core at ~330 GB/s).
Schedule principles:
  - The first DMA wave is exactly the tensors that unblock the PE
    (wqk, qx halves, kx0/kx1); everything else queues behind.  vx3
    lands LAST and feeds the shortest possible post-landing chain
    (per-half V projection accumulation -> 4 transposes -> 16 PV
    matmuls -> div -> store).
  - Each kx piece feeds exactly one scores_pair for block 0 and one
    for block 1; score pairs are emitted in kx-arrival order so the
    serial 12-ACTIVATE exp chain (1.11us each) starts ~12us and never
    starves.
  - PV runs as two PSUM chains per 512-query block ([128, 2, 129]
    pair-tiles, 2 banks total): block 0 completes (divs + store)
    before block 1's chains claim the banks.
  - Warmup/filler matmuls use a 1-column stationary: they keep the HAM
    clock gate open without tripping the sustained-power throttle
    (k=4/8 HAM state halves the PE clock for ~5-7us when it fires).

Layout: the host pre-marshals every tensor into the exact [partition,
chunk, col] block layout the SBUF tiles use (2-8 KB contiguous runs
per partition per DMA).  Every DMA block and projection output gets
its own SBUF tile so tile-granular dependency tracking never
over-serializes.
"""

import math

import numpy as np
import ml_dtypes

import concourse.bacc as bacc
import concourse.tile as tile
import concourse.mybir as mybir
from concourse import bass_utils
from concourse.masks import make_identity
from concourse.tile_rust import add_dep_helper

F32 = mybir.dt.float32
BF16 = mybir.dt.bfloat16
FP8 = mybir.dt.float8e4

B, S, DM, DK = 4, 2048, 1024, 128
NCORES = 8
HALF = S // 2  # query rows per core
NCH = DM // 128  # d_model chunks
# program-wide causal shape: query block 0 sees key tiles [0, NJ0),
# block 1 sees [0, NJ1); per-core mask data zeroes what's invalid.
NJ0, NJ1 = 8, 16
VW = DK + 1  # v tiles carry a ones-column for the softmax denominator
WSC = 16.0  # wq/wk pre-scaled into fp8's normal range
SCALE = 1.0 / (math.sqrt(DK) * WSC * WSC)
WARMUP_MMS = 10
FILLER_MMS = 8

_CACHE = {}


def _build():
    if "nc" in _CACHE:
        return _CACHE["nc"]
    nc = bacc.Bacc("TRN2", target_bir_lowering=False, debug=False, num_devices=NCORES)

    # activations pre-blocked host-side to match SBUF tiles exactly
    qx_in = nc.dram_tensor("qx", [128, 2, NCH, 512], FP8, kind="ExternalInput").ap()
    kx_in = nc.dram_tensor("kx", [128, 8, NCH, 256], FP8, kind="ExternalInput").ap()
    vx_in = nc.dram_tensor("vx", [128, 4, NCH, 512], BF16, kind="ExternalInput").ap()
    wqk_in = nc.dram_tensor("wqk", [128, NCH, 2 * DK], FP8, kind="ExternalInput").ap()
    wv_in = nc.dram_tensor("wv", [128, NCH, DK], BF16, kind="ExternalInput").ap()
    shifts_in = nc.dram_tensor("shifts", [128, 16], F32, kind="ExternalInput").ap()
    out = nc.dram_tensor("out", [HALF, DK], BF16, kind="ExternalOutput").ap()

    with tile.TileContext(nc) as tc:
        with tc.tile_pool(name="const", bufs=1) as const:
            wqk = const.tile([128, NCH, 2 * DK], FP8, tag="wqk", name="wqk")
            wv = const.tile([128, NCH, DK], BF16, tag="wv", name="wv")
            shifts = const.tile([128, 16], F32)
            # qx halves: [block][half] -> chunks 4h..4h+3
            qxh = [[const.tile([128, 4, 512], FP8, tag=f"qx{b}{h}", name=f"qx{b}{h}")
                    for h in range(2)] for b in range(2)]
            kxp = [const.tile([128, NCH, 256], FP8, tag=f"kx{b}", name=f"kx{b}")
                   for b in range(8)]
            # vx: every quarter split into two chunk-half tiles so the
            # projection can consume each half as it lands
            vxh = {(q, h): const.tile([128, 4, 512], BF16, tag=f"vx{q}{h}",
                                      name=f"vx{q}{h}")
                   for q in range(4) for h in range(2)}

            w_warm = const.tile([128, 512], BF16)
            nc.vector.memset(w_warm, 1.0)

            # ---- loads.  Each queue drains at ~110 GB/s, so an item's
            # landing time is ~8.4us + cumulative-bytes-before-it/110GB/s.
            # Queue positions are chosen so every tensor lands just before
            # its consumer needs it; the exp chain (one kx piece per
            # ~2.2us from ~14us) and the vx quarters (one per ~4.5us from
            # ~20us) set the deadlines.  vx3 lands last by design.
            # scalar: 5 up-front (HWDGE ring holds 4; the 5th blocks until
            # wqk completes ~10.7) + vx2 halves issued mid-exp-chain.
            nc.scalar.dma_start(out=wqk, in_=wqk_in)
            nc.scalar.dma_start(out=qxh[0][1], in_=qx_in[:, 0, 4:8])
            nc.scalar.dma_start(out=kxp[2], in_=kx_in[:, 2])
            nc.scalar.dma_start(out=wv, in_=wv_in)
            nc.scalar.dma_start(out=vxh[0, 0], in_=vx_in[:, 0, 0:4])

            nc.sync.dma_start(out=shifts, in_=shifts_in)
            nc.sync.dma_start(out=qxh[0][0], in_=qx_in[:, 0, 0:4])
            nc.sync.dma_start(out=qxh[1][1], in_=qx_in[:, 1, 4:8])
            nc.sync.dma_start(out=kxp[4], in_=kx_in[:, 4])
            nc.sync.dma_start(out=kxp[7], in_=kx_in[:, 7])
            nc.sync.dma_start(out=vxh[0, 1], in_=vx_in[:, 0, 4:8])
            nc.sync.dma_start(out=vxh[1, 1], in_=vx_in[:, 1, 4:8])
            nc.sync.dma_start(out=vxh[3, 0], in_=vx_in[:, 3, 0:4])

            nc.gpsimd.dma_start(out=kxp[0], in_=kx_in[:, 0])
            nc.gpsimd.dma_start(out=kxp[1], in_=kx_in[:, 1])
            nc.gpsimd.dma_start(out=qxh[1][0], in_=qx_in[:, 1, 0:4])
            nc.gpsimd.dma_start(out=kxp[3], in_=kx_in[:, 3])
            nc.gpsimd.dma_start(out=kxp[5], in_=kx_in[:, 5])
            nc.gpsimd.dma_start(out=kxp[6], in_=kx_in[:, 6])
            nc.gpsimd.dma_start(out=vxh[1, 0], in_=vx_in[:, 1, 0:4])
            nc.gpsimd.dma_start(out=vxh[3, 1], in_=vx_in[:, 3, 4:8])

            # gpsimd auxiliary ops AFTER its dma issues (SWDGE descriptor
            # generation runs on the engine and must not be delayed)
            iota_i = const.tile([128, 1024], mybir.dt.int32)
            nc.gpsimd.iota(iota_i[:, 0:512], pattern=[[1, 512]], base=0,
                           channel_multiplier=0)
            nc.gpsimd.iota(iota_i[:, 512:1024], pattern=[[1, 512]], base=-128,
                           channel_multiplier=0)

            ident = const.tile([128, 128], BF16)
            make_identity(nc, ident)

            # ---- causal masks: mask[p, t, c] = (c >= shift[p, t]).
            # iota2's copy is emitted mid-pipeline: it waits on the gpsimd
            # iotas (ready ~15.3us after the lib load) and must not block
            # the kT/qT casts in the in-order vector stream.
            iota2 = const.tile([128, 1024], mybir.dt.float16)
            masks_sb = {}

            def gen_masks(ts):
                for t in ts:
                    masks_sb[t] = const.tile([128, 1024], BF16, tag=f"mask{t}",
                                             name=f"mask{t}")
                    nc.vector.tensor_scalar(
                        masks_sb[t],
                        iota2,
                        shifts[:, t : t + 1],
                        None,
                        op0=mybir.AluOpType.is_ge,
                    )

            # ---- PE warmup + low-power fillers (1-col stationary)
            last_filler = None
            with tc.tile_pool(name="psW", bufs=1, space="PSUM") as psW:
                ps_w = psW.tile([128, 512], F32)
                for _ in range(WARMUP_MMS):
                    nc.tensor.matmul(
                        ps_w[:, 0:128], w_warm[:, 0:128], w_warm[:, 0:128],
                        start=True, stop=True
                    )
                for _ in range(FILLER_MMS):
                    last_filler = nc.tensor.matmul(
                        ps_w[0:1, :], w_warm[:, 0:1], w_warm, start=True, stop=True
                    )

            # ---- projected tensors: one tile per writer
            qTb = [const.tile([128, 512], BF16, tag=f"qT{b}", name=f"qT{b}")
                   for b in range(2)]
            kT = [const.tile([128, 256], BF16, tag=f"kT{b}", name=f"kT{b}")
                  for b in range(8)]
            vTq = [const.tile([128, 512], BF16, tag=f"vT{b}", name=f"vT{b}")
                   for b in range(4)]
            # vsb per quarter: [128, 4 key-tiles, VW]
            vsbq = [const.tile([128, 4, VW], BF16, tag=f"v{q}", name=f"vsb{q}")
                    for q in range(4)]

            with (
                tc.tile_pool(name="psM", bufs=2, space="PSUM") as psM,
                tc.tile_pool(name="psS", bufs=2, space="PSUM") as psS,
                tc.tile_pool(name="psO", bufs=2, space="PSUM") as psO,
                tc.tile_pool(name="pP", bufs=14) as p_pool,
                tc.tile_pool(name="oo", bufs=4) as o_pool,
            ):

                def dep_filler(mm):
                    if last_filler is not None:
                        add_dep_helper(mm.ins, last_filler.ins, sync=False,
                                       reason="run filler first")

                def project_dr(k0, parts, dst, w=512):
                    """fp8 DoubleRow projection of wqk[k0] against chunk-pair
                    list `parts` = [(tile, chunk_lo, n_chunks), ...]."""
                    acc = psM.tile([128, 512], F32, tag="ps_misc", name="acc")
                    u = 0
                    nu = NCH // 2
                    for t, c0, nch in parts:
                        for cc in range(0, nch, 2):
                            mm = nc.tensor.matmul(
                                acc[:, 0:w],
                                wqk[:, c0 + cc : c0 + cc + 2, k0 : k0 + DK],
                                t[:, cc : cc + 2, :],
                                start=(u == 0),
                                stop=(u == nu - 1),
                                perf_mode=mybir.MatmulPerfMode.DoubleRow,
                            )
                            if u == 0:
                                dep_filler(mm)
                            u += 1
                    nc.vector.tensor_copy(dst, acc[:, 0:w])

                def project_v(parts, dst):
                    """bf16 V projection (contract d_model chunks)."""
                    acc = psM.tile([128, 512], F32, tag="ps_misc", name="acc")
                    u = 0
                    for t, c0, nch in parts:
                        for cc in range(nch):
                            mm = nc.tensor.matmul(
                                acc,
                                wv[:, c0 + cc, :],
                                t[:, cc, :],
                                start=(u == 0),
                                stop=(u == NCH - 1),
                            )
                            if u == 0:
                                dep_filler(mm)
                            u += 1
                    nc.vector.tensor_copy(dst, acc)

                def scores_pair(blk, j, masked):
                    """exp(score) for key tiles (j, j+1) x 512 queries of blk."""
                    ps_s = psS.tile([128, 1024], F32, tag="score")
                    for i in range(2):
                        jl = j + i
                        nc.tensor.matmul(
                            ps_s[:, i * 512 : (i + 1) * 512],
                            kT[jl // 2][:, (jl % 2) * 128 : (jl % 2 + 1) * 128],
                            qTb[blk],
                            start=True,
                            stop=True,
                        )
                    p_t = p_pool.tile([128, 1024], BF16, tag="p")
                    nc.scalar.activation(
                        p_t, ps_s, mybir.ActivationFunctionType.Exp, scale=SCALE
                    )
                    if masked:
                        nc.vector.tensor_mul(p_t, p_t, masks_sb[j])
                    return p_t

                def v_quarter(q, parts):
                    """project + transpose vx quarter q into vsbq[q]."""
                    nc.vector.memset(vsbq[q][:, :, DK : DK + 1], 1.0)
                    project_v(parts, vTq[q])
                    ps = psM.tile([128, 4, 128], BF16, tag="ps_misc")
                    for tl in range(4):
                        nc.tensor.transpose(
                            ps[:, tl, :], vTq[q][:, tl * 128 : (tl + 1) * 128],
                            ident
                        )
                    nc.vector.tensor_copy(vsbq[q][:, :, 0:DK], ps)

                o_big = [
                    o_pool.tile([128, 4, DK], BF16, tag=f"ob{b}", name=f"ob{b}",
                                bufs=1)
                    for b in range(2)
                ]
                out4 = out.rearrange("(b p q) k -> b p q k", q=4, p=128)

                def pv(ps_o, p_pairs, qs, jset, start, stop):
                    """accumulate P@[V|1] for query-slice qs over key tiles."""
                    for n, j in enumerate(jset):
                        nc.tensor.matmul(
                            ps_o,
                            p_pairs[(j // 2) * 2][
                                :, (j % 2) * 512 + qs * 128
                                : (j % 2) * 512 + (qs + 1) * 128
                            ],
                            vsbq[j // 4][:, j % 4, :],
                            start=(start and n == 0),
                            stop=(stop and n == len(jset) - 1),
                        )

                def div_out(blk, qs, ps_o):
                    rec = o_pool.tile([128, 1], F32, tag="rec")
                    nc.vector.reciprocal(rec, ps_o[:, DK : DK + 1])
                    nc.vector.tensor_scalar_mul(
                        o_big[blk][:, qs, :], ps_o[:, 0:DK], rec
                    )

                # ---------- pipeline ----------
                # W hints = predicted execution times (us) from the DMA
                # landing model; they shape each engine's static order.
                W = tc.tile_wait_until
                with W(0.0105):
                    project_dr(DK, [(kxp[0], 0, NCH)], kT[0], w=256)
                with W(0.0115):
                    project_dr(0, [(qxh[0][0], 0, 4), (qxh[0][1], 4, 4)], qTb[0])
                with W(0.012):
                    nc.vector.tensor_copy(iota2, iota_i)
                    gen_masks([0, 2])
                p0, p1, p1b = {}, {}, {}
                with W(0.0135):
                    p0[0] = scores_pair(0, 0, True)
                with W(0.014):
                    project_dr(DK, [(kxp[1], 0, NCH)], kT[1], w=256)
                with W(0.0145):
                    project_dr(DK, [(kxp[2], 0, NCH)], kT[2], w=256)
                    gen_masks([4, 6])
                with W(0.015):
                    p0[2] = scores_pair(0, 2, True)
                with W(0.016):
                    project_dr(0, [(qxh[1][0], 0, 4), (qxh[1][1], 4, 4)], qTb[1])
                with W(0.0165):
                    p1[0] = scores_pair(1, 0, False)
                with W(0.017):
                    p0[4] = scores_pair(0, 4, True)
                    # mid-chain scalar issues (land ~27/31)
                    nc.scalar.dma_start(out=vxh[2, 0], in_=vx_in[:, 2, 0:4])
                with W(0.0175):
                    project_dr(DK, [(kxp[3], 0, NCH)], kT[3], w=256)
                    project_dr(DK, [(kxp[7], 0, NCH)], kT[7], w=256)
                with W(0.018):
                    p1[2] = scores_pair(1, 2, False)
                    nc.scalar.dma_start(out=vxh[2, 1], in_=vx_in[:, 2, 4:8])
                with W(0.0188):
                    p0[6] = scores_pair(0, 6, True)
                with W(0.019):
                    project_dr(DK, [(kxp[4], 0, NCH)], kT[4], w=256)
                    gen_masks([8, 10])
                with W(0.020):
                    p1[4] = scores_pair(1, 4, False)
                with W(0.021):
                    project_dr(DK, [(kxp[5], 0, NCH)], kT[5], w=256)
                with W(0.0215):
                    p1[6] = scores_pair(1, 6, False)
                with W(0.022):
                    p1b[8] = scores_pair(1, 8, True)
                    gen_masks([12, 14])
                with W(0.023):
                    v_quarter(0, [(vxh[0, 0], 0, 4), (vxh[0, 1], 4, 4)])
                with W(0.0235):
                    project_dr(DK, [(kxp[6], 0, NCH)], kT[6], w=256)
                with W(0.024):
                    p1b[10] = scores_pair(1, 10, True)
                with W(0.025):
                    p1b[12] = scores_pair(1, 12, True)

                # block 0: PV chains, divs, store (frees psO banks for blk 1)
                ps_o0 = [psO.tile([128, VW], F32, tag="out", name=f"ps_o0_{i}")
                         for i in range(4)]
                with W(0.0255):
                    for qs in range(4):
                        pv(ps_o0[qs], p0, qs, range(NJ0), True, True)
                        div_out(0, qs, ps_o0[qs])
                    nc.sync.dma_start(out=out4[0], in_=o_big[0])
                with W(0.026):
                    p1b[14] = scores_pair(1, 14, True)
                with W(0.027):
                    v_quarter(1, [(vxh[1, 0], 0, 4), (vxh[1, 1], 4, 4)])

                # block 1: one long accumulation per query-slice
                ps_o1 = [psO.tile([128, VW], F32, tag="out", name=f"ps_o1_{i}")
                         for i in range(4)]
                with W(0.030):
                    v_quarter(2, [(vxh[2, 0], 0, 4), (vxh[2, 1], 4, 4)])
                with W(0.0315):
                    v_quarter(3, [(vxh[3, 0], 0, 4), (vxh[3, 1], 4, 4)])
                # per-qs full chains: qs0/1 complete (and store) while qs2/3
                # rotate into the freed PSUM banks
                pall = {**p1, **p1b}
                with W(0.0325):
                    for qs in range(2):
                        pv(ps_o1[qs], pall, qs, range(NJ1), True, True)
                        div_out(1, qs, ps_o1[qs])
                    nc.scalar.dma_start(out=out4[1, :, 0:2], in_=o_big[1][:, 0:2])
                with W(0.0345):
                    for qs in range(2, 4):
                        pv(ps_o1[qs], pall, qs, range(NJ1), True, True)
                        div_out(1, qs, ps_o1[qs])
                    nc.sync.dma_start(out=out4[1, :, 2:4], in_=o_big[1][:, 2:4])

    nc.compile()
    _CACHE["nc"] = nc
    return nc


def _shift_block(h):
    """[128, 16] f32: mask[p, t, c] = (c >= shift) == (key 128t+p <= query qb+c)."""
    qbase = (0, 1536) if h == 0 else (512, 1024)
    p = np.arange(128, dtype=np.float32)[:, None]
    t = np.arange(16, dtype=np.float32)[None, :]
    qb = np.where(t < NJ0, qbase[0], qbase[1])
    return (128.0 * t + p - qb).astype(np.float32)


def _blocked(arr, nblk, dtype):
    """[DM, ncols] -> [128, nblk, NCH, ncols//nblk] matching the SBUF tiles."""
    w = arr.shape[1] // nblk
    return np.ascontiguousarray(
        arr.reshape(NCH, 128, nblk, w).transpose(1, 2, 0, 3)
    ).astype(dtype)


def kernel(**inputs):
    queries = np.asarray(inputs["queries"], dtype=np.float32)
    keys = np.asarray(inputs["keys"], dtype=np.float32)
    values = np.asarray(inputs["values"], dtype=np.float32)

    nc = _build()
    f8 = ml_dtypes.float8_e4m3fn
    bf = ml_dtypes.bfloat16
    shifts = [_shift_block(0), _shift_block(1)]
    qrows = [np.r_[0:512, 1536:2048], np.r_[512:1536]]
    wT = {
        nm: np.asarray(inputs[nm], dtype=np.float32).T
        for nm in ("Wq", "Wk", "Wv")
    }
    wqk = np.ascontiguousarray(
        np.concatenate([wT["Wq"], wT["Wk"]], axis=1).reshape(NCH, 128, 2 * DK)
        .transpose(1, 0, 2) * WSC
    ).astype(f8)
    wv = np.ascontiguousarray(
        wT["Wv"].reshape(NCH, 128, DK).transpose(1, 0, 2)
    ).astype(bf)
    kxs = [_blocked(keys[b].T, 8, f8) for b in range(B)]
    vxs = [_blocked(values[b].T, 4, bf) for b in range(B)]

    in_maps = []
    for c in range(NCORES):
        b, h = c // 2, c % 2
        in_maps.append(
            {
                "qx": _blocked(queries[b][qrows[h]].T, 2, f8),
                "kx": kxs[b],
                "vx": vxs[b],
                "wqk": wqk,
                "wv": wv,
                "shifts": shifts[h],
            }
        )

    res = bass_utils.run_bass_kernel_spmd(
        nc, in_maps, list(range(NCORES)), **_CACHE.get("run_kwargs", {})
    )
    _CACHE["last_result"] = res

    # store layout is (p q): dram row blk*512 + p*4 + qs <- query qs*128 + p
    r = np.arange(512)
    local_q = (r % 4) * 128 + r // 4  # query index within block at dram row r
    perm = np.concatenate([local_q, 512 + local_q])
    out = np.empty((B, S, DK), dtype=np.float32)
    for c in range(NCORES):
        b, h = c // 2, c % 2
        out[b][qrows[h][perm]] = res.results[c]["out"]
    return out
